# revision 16
# baseline (speedup 1.0000x reference)
"""Bass/Trainium2 kernel for nn_BlockGNN (2-layer GATv2 + MLP) on 8 NeuronCores.

Known-good 871us configuration (run2): 128-dst windows, feature-major z,
ACT Prelu |att| trick, per-tile alpha mini-matmuls, per-window software
pipeline, f32 MLP.
"""

import os
import sys
import time

import numpy as np

os.environ.setdefault("MYCRO_LOCAL_CACHE", "1")

for _p in ("/opt/trn_rl_repo",):
    if os.path.isdir(_p) and _p not in sys.path:
        sys.path.append(_p)

import concourse.bass as bass
import concourse.bacc as bacc
import concourse.mybir as mybir
import concourse.tile as tile
from concourse.bass import AP
from concourse.bass_utils import run_bass_kernel_spmd

F32 = mybir.dt.float32
BF16 = mybir.dt.bfloat16
FP8 = mybir.dt.float8e4

NPF32 = np.float32
NPBF16 = mybir.dt.np(BF16)
NPFP8 = mybir.dt.np(FP8)

N, E, D, H, CDIM, L = 50000, 800000, 128, 4, 32, 2
P = 128
NCORES = 8
NEG = 0.2

EDGE_DT = BF16
NP_EDGE = NPBF16
SEL_DT = FP8
NP_SEL = NPFP8
MLP_DT = F32
NP_MLP = NPF32
TRACE = bool(int(os.environ.get("KTRACE", "0")))

LAST_EXEC_NS = []
LAST_RESULTS = []


def _install_ntff_hook():
    try:
        import antenv.axon_hooks  # noqa: F401
        return
    except ImportError:
        pass
    import contextlib
    import ctypes
    import types

    try:
        import antenv
    except ImportError:
        return
    so_path = "/opt/axon/libaxon_pjrt.so"
    if not os.path.exists(so_path):
        return
    lib = ctypes.CDLL(so_path)
    if not hasattr(lib, "axon_start_nrt_profile"):
        return
    lib.axon_start_nrt_profile.argtypes = [
        ctypes.POINTER(ctypes.c_int64),
        ctypes.c_size_t,
    ]
    lib.axon_start_nrt_profile.restype = ctypes.c_int64
    lib.axon_stop_nrt_profile.argtypes = [ctypes.c_char_p]
    lib.axon_stop_nrt_profile.restype = ctypes.c_int64

    @contextlib.contextmanager
    def _hook(output_dir, device_ids):
        import jax

        jax.devices()
        if device_ids:
            ids = (ctypes.c_int64 * len(device_ids))(*device_ids)
            rc = lib.axon_start_nrt_profile(ids, len(device_ids))
        else:
            rc = lib.axon_start_nrt_profile(None, 0)
        if rc != 0:
            raise RuntimeError(f"axon_start_nrt_profile rc={rc}")
        try:
            yield
        finally:
            n = lib.axon_stop_nrt_profile(str(output_dir).encode())
            print(f"ntff profile: {n} file(s) -> {output_dir}", file=sys.stderr)

    mod = types.ModuleType("antenv.axon_hooks")
    _state = {"hook": _hook}
    mod.get_axon_ntff_profile_hook = lambda: _state["hook"]
    mod.set_axon_ntff_profile_hook = lambda h: _state.update(hook=h)
    sys.modules["antenv.axon_hooks"] = mod
    antenv.axon_hooks = mod


if TRACE:
    _install_ntff_hook()


def _bcast_last(ap: AP, n: int) -> AP:
    return AP(ap.tensor, ap.offset, [list(p) for p in ap.ap] + [[0, n]])


def build_layer_nc(cfg, enable_asserts=False):
    NWIN, KT = cfg["nwin"], cfg["kt"]
    NLOCP = NWIN * P
    ESLOT = NWIN * KT * P
    assert KT % 2 == 0
    MACROS = []
    j0 = 0
    while j0 < KT:
        wdt = 4 if KT - j0 >= 4 else KT - j0
        MACROS.append((j0, wdt))
        j0 += wdt
    NMAC = len(MACROS)
    assert NMAC >= 3

    nc = bacc.Bacc(
        "TRN2",
        target_bir_lowering=False,
        debug=False,
        enable_asserts=enable_asserts,
        num_devices=cfg.get("ncores", NCORES),
    )

    xgT = nc.dram_tensor("xgT", [P, ESLOT], EDGE_DT, kind="ExternalInput").ap()
    xTloc = nc.dram_tensor("xTloc", [P, NLOCP], BF16, kind="ExternalInput").ap()
    Wl_b = nc.dram_tensor("Wl_b", [P, P], EDGE_DT, kind="ExternalInput").ap()
    Wr = nc.dram_tensor("Wr", [P, P], BF16, kind="ExternalInput").ap()
    w1 = nc.dram_tensor("w1", [P, P], MLP_DT, kind="ExternalInput").ap()
    w2 = nc.dram_tensor("w2", [P, P], MLP_DT, kind="ExternalInput").ap()
    We_b = nc.dram_tensor("We_b", [CDIM, P], EDGE_DT, kind="ExternalInput").ap()
    attcol = nc.dram_tensor("attcol", [P, 1], F32, kind="ExternalInput").ap()
    pbias = nc.dram_tensor("pbias", [P, 1], F32, kind="ExternalInput").ap()
    sgn4 = nc.dram_tensor("sgn4", [P, 4], BF16, kind="ExternalInput").ap()
    i128f = nc.dram_tensor("i128f", [P, P], F32, kind="ExternalInput").ap()
    b1c = nc.dram_tensor("b1c", [P, 1], F32, kind="ExternalInput").ap()
    b2c = nc.dram_tensor("b2c", [P, 1], F32, kind="ExternalInput").ap()
    bgc = nc.dram_tensor("bgc", [P, 1], F32, kind="ExternalInput").ap()
    eaT = nc.dram_tensor("eaT", [CDIM, ESLOT], EDGE_DT, kind="ExternalInput").ap()
    seld = nc.dram_tensor("seld", [NWIN, P, KT * P], SEL_DT, kind="ExternalInput").ap()
    selTd = nc.dram_tensor("selTd", [NWIN, P, KT * P], SEL_DT, kind="ExternalInput").ap()
    xoutT = nc.dram_tensor("xoutT", [P, NLOCP], F32, kind="ExternalOutput").ap()

    AF = mybir.ActivationFunctionType
    OP = mybir.AluOpType

    with tile.TileContext(nc) as tc:
        with (
            tc.tile_pool(name="const", bufs=1) as cpool,
            tc.tile_pool(name="win", bufs=2) as wpool,
            tc.tile_pool(name="edge", bufs=3) as epool,
            tc.tile_pool(name="psZ", bufs=2, space="PSUM") as psZ,
            tc.tile_pool(name="psV", bufs=3, space="PSUM") as psV,
            tc.tile_pool(name="psM", bufs=1, space="PSUM") as psM,
            tc.tile_pool(name="psO", bufs=1, space="PSUM") as psO,
            tc.tile_pool(name="psE", bufs=1, space="PSUM") as psE,
        ):
            def cload(ap, shape, dt, tag):
                t = cpool.tile(shape, dt, tag=tag)
                nc.sync.dma_start(out=t[:], in_=ap)
                return t

            Wlb_s = cload(Wl_b, [P, P], EDGE_DT, tag="Wlb_s")
            Wr_s = cload(Wr, [P, P], BF16, tag="Wr_s")
            w1_s = cload(w1, [P, P], MLP_DT, tag="w1_s")
            w2_s = cload(w2, [P, P], MLP_DT, tag="w2_s")
            We_s = cload(We_b, [CDIM, P], EDGE_DT, tag="We_s")
            attc_s = cload(attcol, [P, 1], F32, tag="attc_s")
            pbias_s = cload(pbias, [P, 1], F32, tag="pbias_s")
            sgn_s = cload(sgn4, [P, 4], BF16, tag="sgn_s")
            i128f_s = cload(i128f, [P, P], F32, tag="i128f_s")
            b1c_s = cload(b1c, [P, 1], F32, tag="b1c_s")
            b2c_s = cload(b2c, [P, 1], F32, tag="b2c_s")
            bgc_s = cload(bgc, [P, 1], F32, tag="bgc_s")

            pending_epilogue = [None]

            def emit_epilogue():
                fn = pending_epilogue[0]
                if fn is not None:
                    pending_epilogue[0] = None
                    fn()

            for w in range(NWIN):
                xtl_sb = wpool.tile([P, P], BF16, tag="xtl")
                nc.sync.dma_start(out=xtl_sb[:], in_=xTloc[:, w * P : (w + 1) * P])
                xgT_sb = wpool.tile([P, KT * P], EDGE_DT, tag="xgT")
                nc.sync.dma_start(
                    out=xgT_sb[:], in_=xgT[:, w * KT * P : (w + 1) * KT * P]
                )
                eaT_sb = wpool.tile([CDIM, KT * P], EDGE_DT, tag="ea")
                nc.sync.dma_start(
                    out=eaT_sb[:], in_=eaT[:, w * KT * P : (w + 1) * KT * P]
                )
                selT_sb = wpool.tile([P, KT * P], SEL_DT, tag="selT")
                nc.sync.dma_start(out=selT_sb[:], in_=selTd[w])
                sel_sb = wpool.tile([P, KT * P], SEL_DT, tag="sel")
                nc.sync.dma_start(out=sel_sb[:], in_=seld[w])
                xr_ps = psE.tile([P, P], F32, tag="epi")
                nc.tensor.matmul(out=xr_ps[:], lhsT=xtl_sb[:], rhs=Wr_s[:],
                                 start=True, stop=True)
                xr_sb = wpool.tile([P, P], EDGE_DT, tag="xrs")
                nc.vector.tensor_copy(xr_sb[:], xr_ps[:])

                out12 = psO.tile([P, 132], F32, tag="o12")

                state = [None] * NMAC
                pairstate = {}

                def stage_minis(mi):
                    """Minis of macro mi into the pair's shared PSUM tile
                    (one accumulation group per pair, baseline multi-region
                    pattern)."""
                    j0, MW, vq, am = state[mi][:4]
                    pi = mi // 2
                    first = (mi % 2 == 0)
                    last = (mi % 2 == 1) or (mi == NMAC - 1)
                    if first:
                        mini2 = psM.tile([P, 32], F32, tag="mini",
                                         name="mini2")
                        comb2 = epool.tile([P, 8 * 132], EDGE_DT, tag="comb",
                                           name="comb2")
                        pairstate[pi] = dict(mini=mini2, comb2=comb2, boff=0,
                                             members=[])
                    ps = pairstate[pi]
                    off = (mi % 2) * 16
                    for u in range(MW):
                        nc.tensor.matmul(
                            out=ps["mini"][:, off + u * 4 : off + (u + 1) * 4],
                            lhsT=am[:, u * P : (u + 1) * P],
                            rhs=sgn_s[:],
                            start=(first and u == 0),
                            stop=(last and u == MW - 1),
                        )
                    ps["members"].append(mi)
                    state[mi] = state[mi][:4] + (ps, (mi % 2) * 4)

                def stage_expmult(pi):
                    """One Exp over the whole pair, then per-macro DVE mults."""
                    ps = pairstate[pi]
                    members = ps["members"]
                    B = sum(state[m][1] for m in members)
                    comb2 = ps["comb2"]
                    comb_v = comb2[:, : B * 132].rearrange("p (b f) -> p b f",
                                                           f=132)
                    # pair mini layout: member0 in cols 0:16, member1 16:32
                    nc.scalar.activation(
                        comb_v[:, :, P : P + 4],
                        ps["mini"][:, : B * 4].rearrange("p (b h) -> p b h",
                                                         h=4),
                        AF.Exp,
                    )
                    boff = 0
                    for m in members:
                        MW = state[m][1]
                        vq = state[m][2]
                        cseg = comb2[:, boff * 132 : (boff + MW) * 132]
                        cseg_v = cseg.rearrange("p (b f) -> p b f", f=132)
                        nc.vector.tensor_tensor(
                            cseg_v[:, :, 0:P].rearrange(
                                "p b (h c) -> p b h c", c=CDIM),
                            vq[:, : MW * P].rearrange(
                                "p (b h c) -> p b h c", b=MW, c=CDIM),
                            _bcast_last(cseg_v[:, :, P : P + 4], CDIM),
                            op=OP.mult,
                        )
                        state[m] = state[m][:4] + (ps, boff)
                        boff += MW

                def stage_out12(mi):
                    j0, MW, vq, am, ps, boff = state[mi]
                    comb2 = ps["comb2"]
                    for u in range(MW):
                        j = j0 + u
                        nc.tensor.matmul(
                            out=out12[:],
                            lhsT=sel_sb[:, j * P : (j + 1) * P],
                            rhs=comb2[:, (boff + u) * 132
                                      : (boff + u + 1) * 132],
                            start=(mi == 0 and u == 0),
                            stop=(mi == NMAC - 1 and u == MW - 1),
                        )

                for mi, (j0, MW) in enumerate(MACROS):
                    S = MW * P
                    zq = psZ.tile([P, 4 * P], F32, tag="zq")
                    nc.tensor.matmul(
                        out=zq[:, :S],
                        lhsT=Wlb_s[:],
                        rhs=xgT_sb[:, j0 * P : j0 * P + S],
                        start=True, stop=False,
                    )
                    nc.tensor.matmul(
                        out=zq[:, :S],
                        lhsT=We_s[:],
                        rhs=eaT_sb[:, j0 * P : j0 * P + S],
                        start=False, stop=False,
                    )
                    nc.tensor.matmul(
                        out=zq[:, :S],
                        lhsT=xr_sb[:],
                        rhs=selT_sb[:, j0 * P : j0 * P + S],
                        start=False, stop=True,
                    )
                    am = epool.tile([P, 4 * P], EDGE_DT, tag="am")
                    nc.scalar.activation(am[:, :S], zq[:, :S], AF.Prelu,
                                         scale=attc_s[:], bias=pbias_s[:],
                                         alpha=NEG)
                    vq = psV.tile([P, 4 * P], F32, tag="vq")
                    for u in range(MW):
                        j = j0 + u
                        nc.tensor.matmul(
                            out=vq[:, u * P : (u + 1) * P],
                            lhsT=xgT_sb[:, j * P : (j + 1) * P],
                            rhs=Wlb_s[:],
                            start=(u == 0),
                            stop=(u == MW - 1),
                        )
                    state[mi] = (j0, MW, vq, am)

                    if mi == 0:
                        emit_epilogue()
                    if mi >= 1:
                        stage_minis(mi - 1)
                        if (mi - 1) % 2 == 1:
                            stage_expmult((mi - 1) // 2)
                    if mi >= 3 and (mi - 3) % 2 == 0:
                        stage_out12(mi - 3)
                        stage_out12(mi - 2)

                # drain
                stage_minis(NMAC - 1)
                stage_expmult((NMAC - 1) // 2)
                done = NMAC - 3 if (NMAC - 3) % 2 == 0 else NMAC - 2
                for mi in range(done, NMAC):
                    stage_out12(mi)

                def make_epilogue(w, out12):
                    def epi():
                        de = wpool.tile([P, 4], F32, tag="de")
                        nc.vector.tensor_scalar(de[:], out12[:, P : P + 4],
                                                1e-16, None, OP.add)
                        rc = wpool.tile([P, 4], F32, tag="rc")
                        nc.vector.reciprocal(rc[:], de[:])
                        gat = wpool.tile([P, P], F32, tag="gat")
                        for h in range(H):
                            nc.vector.tensor_scalar(
                                gat[:, h * CDIM : (h + 1) * CDIM],
                                out12[:, h * CDIM : (h + 1) * CDIM],
                                rc[:, h : h + 1],
                                None,
                                OP.mult,
                            )
                        gatT_ps = psE.tile([P, P], F32, tag="epi")
                        nc.tensor.transpose(gatT_ps[:], gat[:], i128f_s[:])
                        gTb = wpool.tile([P, P], MLP_DT, tag="gTb")
                        nc.vector.tensor_scalar(gTb[:], gatT_ps[:], bgc_s[:],
                                                None, OP.add)
                        y1_ps = psE.tile([P, P], F32, tag="epi")
                        nc.tensor.matmul(out=y1_ps[:], lhsT=w1_s[:], rhs=gTb[:],
                                         start=True, stop=True)
                        y1s = wpool.tile([P, P], MLP_DT, tag="y1s")
                        nc.scalar.activation(y1s[:], y1_ps[:], AF.Relu,
                                             bias=b1c_s[:])
                        y2_ps = psE.tile([P, P], F32, tag="epi")
                        nc.tensor.matmul(out=y2_ps[:], lhsT=w2_s[:], rhs=y1s[:],
                                         start=True, stop=True)
                        xo = wpool.tile([P, P], F32, tag="xo")
                        nc.vector.tensor_scalar(xo[:], y2_ps[:], b2c_s[:],
                                                None, OP.add)
                        nc.sync.dma_start(out=xoutT[:, w * P : (w + 1) * P],
                                          in_=xo[:])
                    return epi

                pending_epilogue[0] = make_epilogue(w, out12)

            emit_epilogue()

    nc.compile()
    return nc


def _preprocess(edge_index, edge_attr, ncores, nloc, nwin):
    src = np.ascontiguousarray(edge_index[0]).astype(np.int64)
    dst = np.ascontiguousarray(edge_index[1]).astype(np.int64)
    n = nloc * ncores
    ea = np.ascontiguousarray(edge_attr, dtype=np.float32)

    deg = np.bincount(dst, minlength=n).astype(np.float32)
    order = np.argsort(dst, kind="stable")
    dst_s = dst[order]
    src_s = src[order]
    ea_s = ea[order]
    cs = np.concatenate(
        [np.zeros((1, ea.shape[1]), np.float64), np.cumsum(ea_s, 0, dtype=np.float64)]
    )
    starts = np.searchsorted(dst_s, np.arange(n))
    ends = np.searchsorted(dst_s, np.arange(n) + 1)
    loop_attr = ((cs[ends] - cs[starts]) / np.maximum(deg, 1.0)[:, None]).astype(
        np.float32
    )

    import heapq

    cores = []
    maxcnt = 0
    for c in range(ncores):
        base = c * nloc
        lo, hi = starts[base], ends[base + nloc - 1]
        s2 = np.concatenate([src_s[lo:hi], np.arange(base, base + nloc)])
        dl = np.concatenate([dst_s[lo:hi], np.arange(base, base + nloc)]) - base
        e2 = np.concatenate([ea_s[lo:hi], loop_attr[base : base + nloc]], 0)

        w_of = np.empty(nloc, np.int64)
        pos_of = np.empty(nloc, np.int64)
        wdeg = (deg[base : base + nloc] + 1.0).astype(np.int64)
        heap = [(0, w, 0) for w in range(nwin)]
        heapq.heapify(heap)
        for node in np.argsort(-wdeg):
            tot, w, fill = heapq.heappop(heap)
            w_of[node] = w
            pos_of[node] = fill
            fill += 1
            tot += int(wdeg[node])
            if fill < P:
                heapq.heappush(heap, (tot, w, fill))
            else:
                heapq.heappush(heap, (1 << 60, w, fill))
        we = w_of[dl]
        pe_ = pos_of[dl]
        o = np.argsort(we, kind="stable")
        s2, e2, we, pe_ = s2[o], e2[o], we[o], pe_[o]
        wstart = np.searchsorted(we, np.arange(nwin))
        wend = np.searchsorted(we, np.arange(nwin) + 1)
        cnts = wend - wstart
        maxcnt = max(maxcnt, int(cnts.max()))
        nl_flat = np.zeros(nwin * P, np.int64)
        nl_flat[w_of * P + pos_of] = np.arange(nloc)
        used = np.zeros(nwin * P, bool)
        used[w_of * P + pos_of] = True
        cores.append((s2, e2, pe_, wstart, cnts, nl_flat, used))

    kt = -(-maxcnt // P)
    if kt % 2:
        kt += 1
    S = kt * P

    data = []
    for (s2, e2, pe_, wstart, cnts, nl_flat, used) in cores:
        nslot = nwin * S
        src_slot = np.zeros(nslot, np.int64)
        dstw_slot = np.full(nslot, -1, np.int64)
        ea_slot = np.zeros((nslot, CDIM), np.float32)
        idx = np.concatenate([np.arange(cnts[w]) + w * S for w in range(nwin)])
        src_slot[idx] = s2
        dstw_slot[idx] = pe_
        ea_slot[idx] = e2

        dw = dstw_slot.reshape(nwin, kt, P)
        sel = (dw[:, :, :, None] == np.arange(P)[None, None, None, :])
        sel = sel.transpose(0, 2, 1, 3).reshape(nwin, P, kt * P).astype(NP_SEL)
        selT = (dw[:, :, None, :] == np.arange(P)[None, None, :, None])
        selT = selT.transpose(0, 2, 1, 3).reshape(nwin, P, kt * P).astype(NP_SEL)
        eaT = np.ascontiguousarray(ea_slot.T).astype(NP_EDGE)
        data.append(dict(src_slot=src_slot, seld=sel, selTd=selT, eaT=eaT,
                         nl_flat=nl_flat, used=used))
    return data, kt


def _layer_weight_maps(inputs, layer, att):
    i = layer
    attf = att[i].reshape(-1).astype(np.float32)
    sgn = np.zeros((P, H), np.float32)
    for h in range(H):
        sgn[h * CDIM : (h + 1) * CDIM, h] = np.sign(
            attf[h * CDIM : (h + 1) * CDIM]
        )
    m = dict(
        Wl_b=np.ascontiguousarray(inputs["Wl"][i]).astype(NP_EDGE),
        Wr=np.ascontiguousarray(inputs["Wr"][i]).astype(NPBF16),
        w1=np.ascontiguousarray(inputs["w1"][i]).astype(NP_MLP),
        w2=np.ascontiguousarray(inputs["w2"][i]).astype(NP_MLP),
        We_b=np.ascontiguousarray(inputs["We"][i]).astype(NP_EDGE),
        attcol=np.abs(attf).reshape(P, 1).astype(NPF32),
        pbias=(np.abs(attf)
               * (np.asarray(inputs["br"][i]) + np.asarray(inputs["bl"][i])))
        .reshape(P, 1)
        .astype(NPF32),
        sgn4=sgn.astype(NPBF16),
        i128f=np.eye(P, dtype=NPF32),
        b1c=np.asarray(inputs["b1"][i]).reshape(P, 1).astype(NPF32),
        b2c=np.asarray(inputs["b2"][i]).reshape(P, 1).astype(NPF32),
        bgc=(np.asarray(inputs["bias"][i]) + np.asarray(inputs["bl"][i]))
        .reshape(P, 1)
        .astype(NPF32),
    )
    return m


_NC_CACHE = {}


def kernel(**inputs):
    nodes = np.asarray(inputs["nodes"], dtype=np.float32)
    edge_index = np.asarray(inputs["edge_index"])
    edge_attr = np.asarray(inputs["edge_attr"], dtype=np.float32)

    n, d = nodes.shape
    assert (n, d) == (N, D)
    nloc = n // NCORES
    nwin = -(-nloc // P)

    data, kt = _preprocess(edge_index, edge_attr, NCORES, nloc, nwin)

    key = (nwin, kt, NCORES)
    if key not in _NC_CACHE:
        _NC_CACHE[key] = build_layer_nc(dict(nwin=nwin, kt=kt, ncores=NCORES))
    nc = _NC_CACHE[key]

    x_curr = np.ascontiguousarray(nodes.T)

    for layer in range(L):
        wmap = _layer_weight_maps(inputs, layer, np.asarray(inputs["att"]))
        xce = x_curr.astype(NP_EDGE)
        in_maps = []
        for c in range(NCORES):
            base = c * nloc
            xTloc = x_curr[:, base + data[c]["nl_flat"]].copy()
            xTloc[:, ~data[c]["used"]] = 0.0
            m = dict(wmap)
            m["xgT"] = np.ascontiguousarray(xce[:, data[c]["src_slot"]])
            m["xTloc"] = xTloc.astype(NPBF16)
            m["seld"] = data[c]["seld"]
            m["selTd"] = data[c]["selTd"]
            m["eaT"] = data[c]["eaT"]
            in_maps.append(m)
        res = run_bass_kernel_spmd(
            nc, in_maps, core_ids=list(range(NCORES)), trace=TRACE
        )
        if res.exec_time_ns is not None:
            LAST_EXEC_NS.append(res.exec_time_ns)
        if TRACE:
            LAST_RESULTS.append(res)
        outs = res.results
        x_next = np.zeros((P, n), NPF32)
        for c in range(NCORES):
            xo = outs[c]["xoutT"]
            u = data[c]["used"]
            x_next[:, c * nloc + data[c]["nl_flat"][u]] = xo[:, u]
        x_curr = x_next

    return np.ascontiguousarray(x_curr.T.astype(np.float32))
